# revision 31
# baseline (speedup 1.0000x reference)
"""Trainium2 Bass kernel for nn_ProjectionLoss (smooth reprojection loss).

kernel(**inputs) takes FULL unsharded inputs, shards batch B across the 8
NeuronCores (pure data parallel), runs a Bass/Tile kernel per core, and
reduces the per-core per-partition partial sums on the host. Host-side work
is layout only: sharding, a c-major transpose of the keypoints, and packing
[R|t] / cam into one DMA-friendly tensor.

Math (validated vs reference, relerr ~3e-8):
  Mh[b,v,3,4] = K[v] @ [R|t],  Ph[b,v,3,4] = K[v] @ cam
  xg[b,v,j,:] = Mh[b,v] @ [X_gt[b,j],1],  xp likewise with X_pred
  d = xg[..,:2]/xg[..,2:] - xp[..,:2]/xp[..,2:]
  smooth(t) = min(t, 400^0.9 * t^0.1)   == the threshold reweighting
  loss = sum(smooth(d^2)) / (2*B)

Per-core pipeline (B_local=2048, 64 groups of 32 batches):
  stage0 (PE, fp32r): mh[(bl,c4), (s,v,i)] = rt[(s,v,e), (bl,c)].T @ Kblk2
  scatter (DVE/ACT tensor_scalar w/ per-partition 0/1 masks): block-diagonal
    rhs1[(bl,c), (q,v,i)] with column block q <-> partition rows [8q, 8q+8)
  stage1 (PE, fp32r): x[(par,j), (q,v,i)] = xhT[(bl,c), (par,j)].T @ rhs1
    where par = bl%2 and xhT holds [X|1] in a 2-periodic parity layout
  elementwise: reciprocal_approx_fast (z), u = x*r (DVE), d = ug-up (DVE),
    d2 = d^2 (ACT Square), big = exp(0.1*ln(d2) + 0.9*ln(400)) (ACT),
    fused min+sum (DVE tensor_tensor_reduce) -> per-partition accumulators.
"""

import os
import numpy as np

DEBUG_DUMPS = bool(int(os.environ.get("KERNEL_DEBUG_DUMPS", "0")))

B, V, J = 16384, 8, 64
N_CORES = 8
B_LOCAL = B // N_CORES          # 2048
GROUP = 32                      # batches per stage-1 matmul
N_GROUPS = B_LOCAL // GROUP     # 64
QUAD = 4                        # groups per DMA batch
BLK = 8                         # groups per elementwise block
N_BLOCKS = N_GROUPS // BLK      # 8
PAIR2 = 2                       # groups per PSUM x tile
SUPER = 2                       # blocks per smooth/reduce batch

LN_C = float(0.9 * np.log(400.0))

_CACHE = {}


def _emit(tc, nc, aps, repeat=1):
    import concourse.mybir as mybir
    from contextlib import ExitStack

    kg_d, kp_d, rt2_d, kblk_d, qmask_d, out_d, dbg = aps
    f32 = mybir.dt.float32
    f32r = mybir.dt.float32r
    bf16 = mybir.dt.bfloat16
    Alu = mybir.AluOpType
    Act = mybir.ActivationFunctionType

    ctx = ExitStack()
    consts = ctx.enter_context(tc.tile_pool(name="consts", bufs=1))
    rt_pool = ctx.enter_context(tc.tile_pool(name="rt", bufs=3))
    mh_ps_pool = ctx.enter_context(tc.tile_pool(name="mhps", bufs=2, space="PSUM"))
    x_ps_pool = ctx.enter_context(tc.tile_pool(name="xps", bufs=1, space="PSUM"))
    sb_pool = ctx.enter_context(tc.tile_pool(name="work", bufs=2))
    acc_pool = ctx.enter_context(tc.tile_pool(name="acc", bufs=1))

    kblk = consts.tile([48, 48], f32r)
    nc.sync.dma_start(kblk[:], kblk_d)
    qmask = consts.tile([128, 16], f32)
    nc.sync.dma_start(qmask[:], qmask_d)

    acc = acc_pool.tile([128, N_BLOCKS], f32)
    nc.vector.memset(acc[:], 0.0)
    total = acc_pool.tile([128, 1], f32)

    c_tiny = consts.tile([128, 1], f32)
    nc.vector.memset(c_tiny[:], 1e-30)
    c_lnc = consts.tile([128, 1], f32)
    nc.vector.memset(c_lnc[:], LN_C)

    # xh slabs are fully host-packed (zeros, parity ones, (par,c,t) row
    # order) -- one contiguous full-tile DMA per (side, quad).
    xh_pool = ctx.enter_context(tc.tile_pool(name="xh", bufs=3))

    for rep_blk in range(repeat * N_BLOCKS):
        blk = rep_blk % N_BLOCKS
        # ---------- stage 0: mh for the block's 8 groups ----------
        mh_ps = mh_ps_pool.tile([128, 512], f32)
        rt4s = []
        for qd in range(BLK // QUAD):
            quad = blk * (BLK // QUAD) + qd
            rt4 = rt_pool.tile([48, QUAD * 128], f32r, tag="rt", name="rt4")
            nc.sync.dma_start(
                rt4[:], rt2_d[:, quad * QUAD * 128:(quad + 1) * QUAD * 128])
            rt4s.append(rt4)
        for g in range(BLK):
            rt4 = rt4s[g // QUAD]
            gg = g % QUAD
            nc.tensor.matmul(mh_ps[:, 48 * g:48 * g + 48],
                             rt4[:, 128 * gg:128 * gg + 128],
                             kblk[:])
        mh_sb = sb_pool.tile([128, 384], f32r, tag="mhsb")
        nc.scalar.copy(mh_sb[:], mh_ps[:, 0:384])
        if dbg and blk == 0:
            nc.sync.dma_start(dbg["mh"], mh_sb[:])

        # ---------- scatter into block-diagonal rhs1 (both sides at once) ----
        rhs1 = sb_pool.tile([128, 2 * BLK * 384], f32r, tag="rhs",
                            name="rhs1")
        mh_v = mh_sb[:].rearrange("p (g s u) -> p s g u", s=2, u=24)
        rhs_v = rhs1[:].rearrange("p (s g x) -> p s g x", s=2, x=384)
        for q in range(16):
            src = mh_v
            dst = rhs_v[:, :, :, 24 * q:24 * q + 24]
            if q % 8 < 5:
                nc.vector.tensor_scalar_mul(dst, src, qmask[:, q:q + 1])
            else:
                nc.scalar.mul(dst, src, qmask[:, q:q + 1])

        # ---------- xh DMAs for the block's 2 quads ----------
        xhs = {}
        for qd in range(BLK // QUAD):
            quad = blk * (BLK // QUAD) + qd
            for s, src_d in ((0, kg_d), (1, kp_d)):
                xh = xh_pool.tile([128, QUAD * 128], f32r, tag=f"xh{s}",
                                  name=f"xh_{s}")
                nc.sync.dma_start(xh[:], src_d[quad])
                xhs[(qd, s)] = xh

        # ---------- stage 1 + recip/u-mul per 2-group subblock ----------
        u_all = sb_pool.tile([128, BLK * 512], f32, tag="uall",
                             name="u_all")
        for half in range(BLK // PAIR2):
            x_ps = [x_ps_pool.tile([128, 1024], f32, tag=f"xps{s}",
                                   name=f"xps_{s}") for s in range(2)]
            for gg in range(PAIR2):
                g = half * PAIR2 + gg
                for s in range(2):
                    xh = xhs[(g // QUAD, s)]
                    gq = g % QUAD
                    nc.tensor.matmul(
                        x_ps[s][:, 512 * gg:512 * gg + 384],
                        xh[:, 128 * gq:128 * gq + 128],
                        rhs1[:, 3072 * s + 384 * g:3072 * s + 384 * g + 384])
            for s in range(2):
                xq = x_ps[s][:].rearrange("p (g x) -> p g x", x=512)
                xi = xq[:, :, 0:384].rearrange("p g (w i) -> p g w i", i=3)
                z = xi[:, :, :, 2:3].squeeze(3)
                r = sb_pool.tile([128, 256], f32, tag=f"r{s}", name=f"r_{s}")
                nc.vector.reciprocal_approx_fast(
                    r[:].rearrange("p (g w) -> p g w", w=128), z)
                xy = xi[:, :, :, 0:2]
                rb = r[:].rearrange("p (g w) -> p g w", w=128).unsqueeze(
                    3).broadcast_to((128, 2, 128, 2))
                uo = u_all[:].rearrange(
                    "p (hh s x) -> p hh s x", hh=4, s=2)[
                    :, half, s, :].rearrange("p (g w i) -> p g w i", w=128, i=2)
                nc.vector.tensor_tensor(uo, xy, rb, Alu.mult)

        d_bf = sb_pool.tile([128, BLK * 256], bf16, tag="dbf")
        u_v = u_all[:].rearrange("p (h s x) -> p h s x", h=4, s=2)
        nc.vector.tensor_tensor(
            d_bf[:].rearrange("p (h x) -> p h x", h=4),
            u_v[:, :, 0], u_v[:, :, 1], Alu.subtract)
        sb_half = blk % SUPER
        if sb_half == 0:
            d2_all = sb_pool.tile([128, SUPER * BLK * 256], bf16, tag="d2all",
                                  name="d2_all")
        nc.scalar.square(d2_all[:, sb_half * 2048:sb_half * 2048 + 2048],
                         d_bf[:])
        if sb_half == SUPER - 1:
            t_bf = sb_pool.tile([128, SUPER * BLK * 256], bf16, tag="tbf")
            nc.scalar.activation(t_bf[:], d2_all[:], Act.Ln, bias=c_tiny[:])
            big_bf = sb_pool.tile([128, SUPER * BLK * 256], bf16, tag="bigbf")
            nc.scalar.activation(big_bf[:], t_bf[:], Act.Exp, bias=c_lnc[:],
                                 scale=0.1)
            scrap = sb_pool.tile([128, SUPER * BLK * 256], bf16, tag="scrap")
            nc.vector.scalar_tensor_tensor(
                out=scrap[:], in0=d2_all[:], scalar=1.0, in1=big_bf[:],
                op0=Alu.mult, op1=Alu.min,
                accum_out=acc[:, blk // SUPER:blk // SUPER + 1])

    nc.vector.tensor_reduce(total[:], acc[:], mybir.AxisListType.X, Alu.add)
    nc.sync.dma_start(out_d, total[:])
    ctx.close()


def build_module(repeat=1):
    key = ("nc", repeat)
    if key in _CACHE:
        return _CACHE[key]
    import concourse.bacc as bacc
    import concourse.tile as tile
    import concourse.mybir as mybir

    f32 = mybir.dt.float32
    f32r = mybir.dt.float32r
    nc = bacc.Bacc("TRN2", target_bir_lowering=False, debug=False,
                   num_devices=N_CORES)
    dbg = None
    if DEBUG_DUMPS:
        dbg = {
            "mh": nc.dram_tensor("dbg_mh", [128, 384], f32r, kind="ExternalOutput").ap(),
            "rhs0": nc.dram_tensor("dbg_rhs0", [128, BLK * 384], f32r, kind="ExternalOutput").ap(),
            "xps0": nc.dram_tensor("dbg_xps0", [128, 1024], f32, kind="ExternalOutput").ap(),
            "xps1": nc.dram_tensor("dbg_xps1", [128, 1024], f32, kind="ExternalOutput").ap(),
            "u0": nc.dram_tensor("dbg_u0", [128, BLK * 256], f32, kind="ExternalOutput").ap(),
            "u1": nc.dram_tensor("dbg_u1", [128, BLK * 256], f32, kind="ExternalOutput").ap(),
        }
    aps = (
        nc.dram_tensor("kg", [N_GROUPS // QUAD, 128, QUAD * 128], f32r, kind="ExternalInput").ap(),
        nc.dram_tensor("kp", [N_GROUPS // QUAD, 128, QUAD * 128], f32r, kind="ExternalInput").ap(),
        nc.dram_tensor("rt2", [48, B_LOCAL * 4], f32r, kind="ExternalInput").ap(),
        nc.dram_tensor("kblk", [48, 48], f32r, kind="ExternalInput").ap(),
        nc.dram_tensor("qmask", [128, 16], f32, kind="ExternalInput").ap(),
        nc.dram_tensor("out", [128, 1], f32, kind="ExternalOutput").ap(),
        dbg,
    )
    with tile.TileContext(nc) as tc:
        _emit(tc, nc, aps, repeat=repeat)
    nc.compile()
    _CACHE[key] = nc
    return nc


def host_consts(K):
    """kblk2 [48,48]: Kblk2[24s+3v+e, 3v+i] = K[v,i,e]; qmask [128,16]."""
    kblk = np.zeros((48, 48), np.float32)
    for s in range(2):
        for v in range(V):
            kblk[24 * s + 3 * v:24 * s + 3 * v + 3,
                 24 * s + 3 * v:24 * s + 3 * v + 3] = K[v].T.astype(np.float32)
    qmask = np.zeros((128, 16), np.float32)
    for p in range(128):
        qmask[p, p % 16] = 1.0
    return kblk, qmask


def make_in_maps(inputs):
    kblk, qmask = host_consts(np.asarray(inputs["K"], np.float32))
    kg = np.asarray(inputs["kps_world_gt"], np.float32)
    kp = np.asarray(inputs["kps_world_pred"], np.float32)
    R = np.asarray(inputs["gt_R"], np.float32)
    t = np.asarray(inputs["gt_t"], np.float32)
    cam = np.asarray(inputs["cam_preds"], np.float32)
    def pack_xh(x):
        # x [B, J, 3] -> slabs [NQ, 128, QUAD*128] with
        # slab[q, 64par+16c+t, 128g+64par+j] = [x|1][32*(4q+g)+2t+par, j, c]
        Bl = x.shape[0]
        nq = Bl // (QUAD * GROUP)
        x4 = np.concatenate([x, np.ones_like(x[..., :1])], -1)  # [Bl, J, 4]
        v = x4.reshape(nq, QUAD, 16, 2, J, 4)       # q g t par j c
        arr = v.transpose(0, 3, 5, 2, 1, 4)         # q par c t g j
        xh = np.zeros((nq, 128, QUAD * 128), np.float32)
        xhv = xh.reshape(nq, 2, 4, 16, QUAD, 2, J)  # q par_r c t g par_c j
        for par in range(2):
            xhv[:, par, :, :, :, par, :] = arr[:, par]
        return xh
    kg_t = np.concatenate([pack_xh(kg[c * B_LOCAL:(c + 1) * B_LOCAL])
                           for c in range(N_CORES)])  # [8*NQ, 128, 512]
    kp_t = np.concatenate([pack_xh(kp[c * B_LOCAL:(c + 1) * B_LOCAL])
                           for c in range(N_CORES)])
    Rt = np.concatenate([R, t[..., None]], axis=-1)      # [B, V, 3, 4]
    in_maps = []
    for c in range(N_CORES):
        sl = slice(c * B_LOCAL, (c + 1) * B_LOCAL)
        # rt2[24s+3v+e, 4b+c] = (Rt if s==0 else cam)[b, v, e, c]
        # col order within each 32-batch group: 64par + 16c + t
        def pack(x):
            xg = x.reshape(N_GROUPS, 16, 2, V, 3, 4)       # g t u v e c
            return np.ascontiguousarray(
                xg.transpose(3, 4, 0, 2, 5, 1)).reshape(24, B_LOCAL * 4)
        rt2 = np.concatenate([pack(Rt[sl]), pack(cam[sl])], axis=0)
        nq = N_GROUPS // QUAD
        in_maps.append({
            "kg": np.ascontiguousarray(kg_t[c * nq:(c + 1) * nq]),
            "kp": np.ascontiguousarray(kp_t[c * nq:(c + 1) * nq]),
            "rt2": np.ascontiguousarray(rt2),
            "kblk": kblk,
            "qmask": qmask,
        })
    return in_maps


def kernel(**inputs):
    from concourse.bass_utils import run_bass_kernel_spmd
    nc = build_module()
    in_maps = make_in_maps(inputs)
    res = run_bass_kernel_spmd(nc, in_maps, list(range(N_CORES)))
    parts = [r["out"] for r in res.results]
    tot = np.sum(np.stack(parts).astype(np.float64))
    return np.float32(tot / (2.0 * B))
